# revision 24
# baseline (speedup 1.0000x reference)
"""Fixed-point attention (nn_Attention_17918603559191) on 8 TRN2 NeuronCores.

Sharding (no collectives): core c -> batch b = c//2, token-half t = c%2.
Each core computes K,V for all 2048 tokens of its batch and Q/attention/
proj for its 1024 tokens. The host rotates the token axis of x per core so
the core's q-tokens are always columns 0..1023 (identical SPMD program);
attention is invariant to permuting the key/value axis.

v5: QUERY-MAJOR softmax, fully interleaved schedule.
 - dots computed [128 q, 2048 k] per (head, q-chunk); exp on ACT with
   accum_out -> softmax denominator S per q-token for free (fp32
   accumulate); the softmax-sum ones-matmul is gone from PE (~102us).
 - rb = 4096/S is per-partition -> quantize is ONE fused DVE pass
   (eb*rb + qshift -> int16, 4x mode) + int16->fp16 cast (4x).
 - attn transposed back to key-major for the AV matmul via DMA XBAR
   transpose in k-halves (idle DMA engines; AV starts on strips 0..7
   while 8..15 still transpose).
 - all f32->f16 input casts ride on gpsimd casting DMAs (no engine
   passes, no staging buffers); Pool queue order: xT, first K/Q weight
   strips, wv, wproj, then per-group weight strips.
 - K/Q strips for head-pair hp+1 and V columns for hp+1 are emitted as
   22 filler chunks spread between the 16 attention steps of head-pair
   hp, so the qkv matmuls hide under the softmax pipeline.
 - per-step rb chain (S-halves add, reciprocal, x4096) on DVE; Pool
   carries only DMA desc-gen (it blocks in-order behind big casting
   DMAs, so nothing latency-critical lives there).

Numerics vs reference (floor onto the 1/4096 grid): S fp32-exact, rb
fp32 reciprocal, floor via fp32 ALU mult-add -> int16. HW's f32->i16
conversion rounds to nearest, so -0.499 implements floor; CoreSim's
truncates (set K_QSHIFT=0 there). q/k/v/ao stored fp16 round-to-nearest
instead of grid truncation. HW-validated rel err 9.82e-3 (< 2e-2 gate,
baseline was 1.066e-2).
"""
import sys

sys.path.insert(0, "/opt/trn_rl_repo")

import os
import numpy as np
import concourse.bass as bass
import concourse.tile as tile
from concourse import mybir, bacc
from concourse.bass_utils import run_bass_kernel_spmd

F32 = mybir.dt.float32
F16 = mybir.dt.float16
I16 = mybir.dt.int16
ALU = mybir.AluOpType
AF = mybir.ActivationFunctionType

D = 1024      # model dim
M = 2048      # key/value tokens per core (full batch)
NQ = 1024     # query tokens per core
H = 16
DH = 64
HP = H // 2   # head pairs (two heads share a 128-row strip)
GRID = 4096.0

K_NEWTON = os.environ.get("K_NEWTON", "0") == "1"
# HW DVE float->int16 conversion rounds to nearest, so -0.499 implements
# floor(y). CoreSim's conversion truncates instead; set K_QSHIFT=0 there.
K_QSHIFT = float(os.environ.get("K_QSHIFT", "-0.499"))

_CACHED_NC = None


def build_kernel(reps=1):
    nc = bacc.Bacc(name="fxp_attn")
    xT_e = nc.declare_dram_parameter("xT", [D, M], F32, isOutput=False)
    wqkvT_e = nc.declare_dram_parameter("wqkvT", [D, 3 * D], F32, isOutput=False)
    wprojT_e = nc.declare_dram_parameter("wprojT", [D, D], F32, isOutput=False)
    bias_e = nc.declare_dram_parameter("bias", [1, D], F32, isOutput=False)
    out_e = nc.declare_dram_parameter("out", [D, NQ], F32, isOutput=True)

    with tile.TileContext(nc) as tc:
        from contextlib import ExitStack
        with ExitStack() as ctx:
            persist = ctx.enter_context(tc.tile_pool(name="persist", bufs=1))

            bias_sb = persist.tile([128, 8], F32, tag="bias")
            nc.sync.dma_start(out=bias_sb, in_=bass.AP(
                tensor=bias_e.ap().tensor, offset=0, ap=[[1, 128], [128, 8]]))

            # persistent fp16 activations
            k_s = [persist.tile([128, M], F16, tag=f"k{s}", name=f"k{s}")
                   for s in range(8)]
            q_s = [persist.tile([128, NQ], F16, tag=f"q{s}", name=f"q{s}")
                   for s in range(8)]
            v_t = [persist.tile([128, D], F16, tag=f"v{t}", name=f"v{t}")
                   for t in range(16)]
            ao_s = [persist.tile([128, NQ], F16, tag=f"ao{s}", name=f"ao{s}")
                    for s in range(8)]

            for _rep in range(reps):
                _run_phases(nc, tc, bias_sb, k_s, q_s, v_t, ao_s,
                            xT_e, wqkvT_e, wprojT_e, out_e)

    nc.compile()
    return nc


def _run_phases(nc, tc, bias_sb, k_s, q_s, v_t, ao_s,
                xT_e, wqkvT_e, wprojT_e, out_e):
    from contextlib import ExitStack
    with ExitStack() as ctx:
        ph1 = ctx.enter_context(tc.tile_pool(name="ph1", bufs=1))
        wfp = ctx.enter_context(tc.tile_pool(name="wfp", bufs=3))
        ps1 = ctx.enter_context(tc.tile_pool(name="ps1", bufs=2, space="PSUM"))
        ebp = ctx.enter_context(tc.tile_pool(name="ebp", bufs=3))
        aip = ctx.enter_context(tc.tile_pool(name="aip", bufs=3))
        afp = ctx.enter_context(tc.tile_pool(name="afp", bufs=4))
        aftp = ctx.enter_context(tc.tile_pool(name="aftp", bufs=4))
        spool = ctx.enter_context(tc.tile_pool(name="sp", bufs=4))
        dotp = ctx.enter_context(tc.tile_pool(name="dotp", bufs=2, space="PSUM"))
        avp = ctx.enter_context(tc.tile_pool(name="avp", bufs=2, space="PSUM"))

        # ---- input loads: gpsimd casting DMAs (f32 DRAM -> f16 SBUF) ----
        # Pool-queue order matters: xT quarters, then the first K/Q weight
        # strips (so PE can start), then wv, then the rest.
        xT = ph1.tile([128, 8, M], F16, tag="xT")
        for tq in range(4):
            nc.gpsimd.dma_start(
                out=xT[:, :, tq * 512:(tq + 1) * 512],
                in_=bass.AP(tensor=xT_e.ap().tensor, offset=tq * 512,
                            ap=[[M, 128], [M * 128, 8], [1, 512]]))

        def stream_w(col0):
            """wqkvT[:, col0:col0+128] -> f16 [128, 8, 128] via casting DMA."""
            wf = wfp.tile([128, 8, 128], F16, tag="wf")
            nc.gpsimd.dma_start(out=wf, in_=bass.AP(
                tensor=wqkvT_e.ap().tensor, offset=col0,
                ap=[[3 * D, 128], [3 * D * 128, 8], [1, 128]]))
            return wf

        wf_k0 = stream_w(D + 0 * 128)
        wf_q0 = stream_w(0 * 128)

        wv = ph1.tile([128, 8, D], F16, tag="wv")
        nc.gpsimd.dma_start(
            out=wv,
            in_=bass.AP(tensor=wqkvT_e.ap().tensor, offset=2 * D,
                        ap=[[3 * D, 128], [3 * D * 128, 8], [1, D]]))

        wp_s = []
        for s in range(8):
            wp = ph1.tile([128, D], F16, tag=f"wp{s}")
            nc.gpsimd.dma_start(
                out=wp, in_=wprojT_e.ap()[s * 128:(s + 1) * 128, :])
            wp_s.append(wp)

        def emit_k_chunk(s, wf, mc):
            pt = ps1.tile([128, 512], F32, tag="ps1")
            for dt in range(8):
                nc.tensor.matmul(
                    pt, lhsT=wf[:, dt, :],
                    rhs=xT[:, dt, mc * 512:(mc + 1) * 512],
                    start=(dt == 0), stop=(dt == 7))
            nc.vector.tensor_scalar(k_s[s][:, mc * 512:(mc + 1) * 512],
                                    pt, 1.0, None, op0=ALU.mult)

        def emit_q_chunk(s, wf, mc):
            pt = ps1.tile([128, 512], F32, tag="ps1")
            for dt in range(8):
                nc.tensor.matmul(
                    pt, lhsT=wf[:, dt, :],
                    rhs=xT[:, dt, mc * 512:(mc + 1) * 512],
                    start=(dt == 0), stop=(dt == 7))
            nc.vector.tensor_scalar(q_s[s][:, mc * 512:(mc + 1) * 512],
                                    pt, 1.0, None, op0=ALU.mult)

        def emit_v_chunk(hp, ts):
            # V columns for head-pair hp only: [128 tok, 128 inner]
            pt = ps1.tile([128, 512], F32, tag="ps1")
            for dt in range(8):
                nc.tensor.matmul(
                    pt[:, 0:128], lhsT=xT[:, dt, ts * 128:(ts + 1) * 128],
                    rhs=wv[:, dt, hp * 128:(hp + 1) * 128],
                    start=(dt == 0), stop=(dt == 7))
            nc.vector.tensor_scalar(v_t[ts][:, hp * 128:(hp + 1) * 128],
                                    pt[:, 0:128], 1.0, None, op0=ALU.mult)

        # ---- attention step (query-major) ----
        def step(hp, h, qs):
            p0 = h * 64
            n0 = qs * 128
            eb = ebp.tile([128, M], F16, tag="eb")
            S2 = spool.tile([128, 2], F32, tag="S2")
            for kh in range(2):
                dt_ps = dotp.tile([128, 2, 512], F32, tag="dt")
                for kc in range(2):
                    nc.tensor.matmul(
                        dt_ps[:, kc, :],
                        lhsT=q_s[hp][p0:p0 + 64, n0:n0 + 128],
                        rhs=k_s[hp][p0:p0 + 64,
                                    (2 * kh + kc) * 512:(2 * kh + kc + 1) * 512],
                        start=True, stop=True,
                        tile_position=(p0, 0))
                nc.scalar.activation(eb[:, kh * 1024:(kh + 1) * 1024], dt_ps,
                                     AF.Exp, scale=0.125,
                                     accum_out=S2[:, kh:kh + 1])
            # rb4 = 4096/S (fp32 per-partition), all on DVE: the Pool queue
            # carries big casting-DMA desc-gen and must stay off this
            # latency-critical chain.
            Sv = spool.tile([128, 1], F32, tag="Sv")
            nc.vector.tensor_tensor(Sv, S2[:, 0:1], S2[:, 1:2], op=ALU.add)
            r0 = spool.tile([128, 1], F32, tag="r0")
            with nc.allow_low_precision(reason="fp32 recip; baseline used fp16"):
                nc.vector.reciprocal(r0, Sv)
            rb4 = spool.tile([128, 1], F32, tag="rb4")
            if K_NEWTON:
                t1 = spool.tile([128, 1], F32, tag="t1")
                nc.vector.tensor_tensor(t1, Sv, r0, op=ALU.mult)
                t2 = spool.tile([128, 1], F32, tag="t2")
                nc.vector.tensor_scalar(t2, t1, -1.0, 2.0, op0=ALU.mult,
                                        op1=ALU.add)
                t3 = spool.tile([128, 1], F32, tag="t3")
                nc.vector.tensor_tensor(t3, t2, r0, op=ALU.mult)
                nc.vector.tensor_scalar(rb4, t3, GRID, None, op0=ALU.mult)
            else:
                nc.vector.tensor_scalar(rb4, r0, GRID, None, op0=ALU.mult)
            # fused quantize: ai = (eb * rb4) + qshift -> int16 (floor on HW)
            ai = aip.tile([128, M], I16, tag="ai")
            nc.vector.tensor_scalar(ai, eb, rb4, K_QSHIFT,
                                    op0=ALU.mult, op1=ALU.add)
            af = afp.tile([128, M], F16, tag="af")
            afT = aftp.tile([128, 16, 128], F16, tag="afT")
            for kh in range(2):
                nc.vector.tensor_scalar(af[:, kh * 1024:(kh + 1) * 1024],
                                        ai[:, kh * 1024:(kh + 1) * 1024],
                                        1.0, None, op0=ALU.mult)
                # XBAR transpose -> key-major [128 k, 8 mt, 128 q] per half,
                # so the AV can start on strips 0..7 while half 1 transposes
                nc.sync.dma_start(out=afT[:, kh * 8:(kh + 1) * 8, :],
                                  in_=af[:, kh * 1024:(kh + 1) * 1024],
                                  transpose=True)
            # AV: accumulate over key strips; out [64 dh, 128 q]
            av = avp.tile([128, 128], F32, tag="av")
            for mt in range(16):
                nc.tensor.matmul(
                    av[p0:p0 + 64, :],
                    lhsT=v_t[mt][:, (2 * hp + h) * 64:(2 * hp + h + 1) * 64],
                    rhs=afT[:, mt, :],
                    start=(mt == 0), stop=(mt == 15),
                    tile_position=(0, p0))
            # evac on DVE (Pool cannot read PSUM): natural units
            nc.vector.tensor_scalar(ao_s[hp][p0:p0 + 64, n0:n0 + 128],
                                    av[p0:p0 + 64, :], 1.0 / GRID, None,
                                    op0=ALU.mult)

        # ---- interleaved schedule ----
        # head: K0/Q0 strips + V columns for hp=0, then per-group steps with
        # next group's K/Q/V chunks sprinkled between steps.
        for mc in range(4):
            emit_k_chunk(0, wf_k0, mc)
        for mc in range(2):
            emit_q_chunk(0, wf_q0, mc)
        for ts in range(16):
            emit_v_chunk(0, ts)

        for hp in range(HP):
            filler = []
            if hp + 1 < HP:
                wf_k = stream_w(D + (hp + 1) * 128)
                wf_q = stream_w((hp + 1) * 128)
                filler += [lambda mc=mc, w=wf_k: emit_k_chunk(hp + 1, w, mc)
                           for mc in range(4)]
                filler += [lambda mc=mc, w=wf_q: emit_q_chunk(hp + 1, w, mc)
                           for mc in range(2)]
                filler += [lambda ts=ts: emit_v_chunk(hp + 1, ts)
                           for ts in range(16)]
            fi = 0
            for i, (h, qs) in enumerate([(h, qs) for h in range(2)
                                         for qs in range(8)]):
                step(hp, h, qs)
                want = min(len(filler), (i + 1) * len(filler) // 10)
                while fi < want:
                    filler[fi]()
                    fi += 1

    # ---------------- Phase 3: projection --------------------------
    with tc.tile_pool(name="ps3", bufs=4, space="PSUM") as ps3, \
         tc.tile_pool(name="outp", bufs=2) as outp:
        for ds in range(8):
            for ch in range(2):
                pt = ps3.tile([128, 512], F32, tag="ps3")
                for es in range(8):
                    nc.tensor.matmul(
                        pt, lhsT=wp_s[es][:, ds * 128:(ds + 1) * 128],
                        rhs=ao_s[es][:, ch * 512:(ch + 1) * 512],
                        start=(es == 0), stop=(es == 7))
                ot = outp.tile([128, 512], F32, tag="ot")
                nc.vector.tensor_scalar(ot, pt, bias_sb[:, ds:ds + 1], None,
                                        op0=ALU.add)
                nc.sync.dma_start(
                    out=out_e.ap()[ds * 128:(ds + 1) * 128,
                                   ch * 512:(ch + 1) * 512],
                    in_=ot)


def _get_nc():
    global _CACHED_NC
    if _CACHED_NC is None:
        _CACHED_NC = build_kernel()
    return _CACHED_NC


def prep(inputs):
    """Build (nc, in_maps) for the 8 cores from full inputs."""
    x, w_qkv, w_proj, b_proj = (inputs["x"], inputs["w_qkv"],
                                inputs["w_proj"], inputs["b_proj"])
    nc = _get_nc()
    wqkvT = np.ascontiguousarray(w_qkv.astype(np.float32).T)
    wprojT = np.ascontiguousarray(w_proj.astype(np.float32).T)
    bias = b_proj.astype(np.float32).reshape(1, D)

    in_maps = []
    for c in range(8):
        b, t = c // 2, c % 2
        xb = x[b].astype(np.float32)
        xrot = np.concatenate([xb[t * NQ:], xb[:t * NQ]], axis=0)
        in_maps.append({
            "xT": np.ascontiguousarray(xrot.T),
            "wqkvT": wqkvT,
            "wprojT": wprojT,
            "bias": bias,
        })
    return nc, in_maps


def kernel(x, w_qkv, w_proj, b_proj, **_):
    B, N, Dm = x.shape
    assert (B, N, Dm) == (4, 2048, 1024)
    nc, in_maps = prep({"x": x, "w_qkv": w_qkv, "w_proj": w_proj,
                        "b_proj": b_proj})

    # First execution after a fresh compile has (rarely) returned stale
    # data through the axon/PJRT path; run twice and keep the second.
    run_bass_kernel_spmd(nc, in_maps, list(range(8)))
    res = run_bass_kernel_spmd(nc, in_maps, list(range(8)))
    global LAST_RESULT
    LAST_RESULT = res
    out = np.empty((B, N, Dm), dtype=np.float32)
    for c in range(8):
        b, t = c // 2, c % 2
        out[b, t * NQ:(t + 1) * NQ, :] = res.results[c]["out"].T
    return out
